# revision 9
# baseline (speedup 1.0000x reference)
"""Distributed Trainium2 kernel for the sparse-attention + depthwise-conv module.

Math: q = l2norm_n(x@Wq), k = l2norm_n(x@Wk) projected per head; the score
matrix attn = softmax(k q^T) is tiny [b,h,64,64].  Since scores and the q/k
norms are bilinear in x through G = X^T X (per-batch [64,64] Gram matrix), the
whole attention output collapses to out = X @ Weff(G) with
  S_raw[h] = Wk_h^T G Wq_h,  kk = diag(Wk_h^T G Wk_h), qq = diag(Wq_h^T G Wq_h)
  attn = softmax(S_raw * rescale / sqrt(kk qq)),
  Wtilde[h] = attn_h^T (Wp_h / rowsum),  Weff = Wv @ Wtilde.
Only G needs cross-core reduction (AllReduce of 2*64*64 f32 = 32KB).

Sharding: spatial rows h=256 split into 8 slabs of 32 rows; both batches on
every core.  Each core computes its G partial, AllReduces, then one fused pass
out[c, n] = sum_taps tap^T x_shift + X@Weff + bp where the 3x3 depthwise conv
taps are diagonal matrices; vertical tap pairs are packed to K=128 via a
row-shifted copy of x in partitions 64:128, the attention term rides the
center tap, and the two batches run concurrently in separate PE column groups
(tile_position col 0/64).
"""

import os
import numpy as np
import ml_dtypes

BF = ml_dtypes.bfloat16
B, C, H, W = 2, 64, 256, 256
HEADS, D = 8, 64
INNER = HEADS * D          # 512
NCORES = 8
RPC = H // NCORES          # 32 output rows per core per batch
WP = W + 2                 # 258 padded row length
HP = RPC + 2               # 34 rows incl halo
FREE = HP * WP             # 8772
SHIFT_FREE = FREE - WP     # 8514
NLOC = RPC * W             # 8192 spatial positions per core per batch
NCHUNK = 512
NCHUNKS = NLOC // NCHUNK   # 16

_CACHE = {}


def _build():
    import concourse.bass as bass
    import concourse.bacc as bacc
    import concourse.mybir as mybir
    import concourse.tile as tile

    f32 = mybir.dt.float32
    bf16 = mybir.dt.bfloat16

    nc = bacc.Bacc("TRN2", target_bir_lowering=False, debug=False,
                   num_devices=NCORES)

    x_d = nc.dram_tensor("x", [B * C, FREE], f32, kind="ExternalInput").ap()
    wq_d = nc.dram_tensor("wq", [C, INNER], bf16, kind="ExternalInput").ap()
    wk_d = nc.dram_tensor("wk", [C, INNER], bf16, kind="ExternalInput").ap()
    wvt_d = nc.dram_tensor("wvt", [128, 256], bf16, kind="ExternalInput").ap()
    wp_d = nc.dram_tensor("wp", [D, INNER], f32, kind="ExternalInput").ap()
    taps_d = nc.dram_tensor("taps", [128, 192], bf16, kind="ExternalInput").ap()
    taps2_d = nc.dram_tensor("taps2", [C, 192], bf16, kind="ExternalInput").ap()
    ones_d = nc.dram_tensor("ones", [C, 1], bf16, kind="ExternalInput").ap()
    iden_d = nc.dram_tensor("iden", [C, C], bf16, kind="ExternalInput").ap()
    bp_d = nc.dram_tensor("bp", [128, 1], f32, kind="ExternalInput").ap()
    rsc_d = nc.dram_tensor("rsc", [1, INNER], bf16, kind="ExternalInput").ap()
    out_d = nc.dram_tensor("out", [B * C, NLOC], f32, kind="ExternalOutput").ap()

    Act = mybir.ActivationFunctionType

    with tile.TileContext(nc) as tc:
        with (
            tc.tile_pool(name="xp", bufs=1) as xpool,
            tc.tile_pool(name="wp", bufs=1) as wpool,
            tc.tile_pool(name="sp", bufs=1) as spool,
            tc.tile_pool(name="xt", bufs=2) as xtpool,
            tc.tile_pool(name="ob", bufs=2) as opool,
            tc.tile_pool(name="ps", bufs=1, space="PSUM") as pspool,
            tc.tile_pool(name="dr", bufs=1, space="DRAM") as drpool,
        ):
            # ---- load x (cast f32->bf16 in flight), build row-shifted copies
            x0 = xpool.tile([128, FREE], bf16, tag="x0")
            x1 = xpool.tile([128, FREE], bf16, tag="x1")
            nc.gpsimd.dma_start(x0[0:64, :], x_d[0:64, :])
            nc.gpsimd.dma_start(x1[0:64, :], x_d[64:128, :])
            nc.vector.tensor_copy(x0[64:128, 0:SHIFT_FREE], x0[0:64, WP:FREE])
            nc.vector.tensor_copy(x1[64:128, 0:SHIFT_FREE], x1[0:64, WP:FREE])

            # ---- weights
            wq_s = wpool.tile_from(wq_d)
            wk_s = wpool.tile_from(wk_d)
            wvt_s = wpool.tile_from(wvt_d)
            wp_s = wpool.tile_from(wp_d)
            taps_s = wpool.tile_from(taps_d)
            taps2_s = wpool.tile_from(taps2_d)
            ones_s = wpool.tile_from(ones_d)
            iden_s = wpool.tile_from(iden_d)
            bp_s = wpool.tile_from(bp_d)
            rsc_s = wpool.tile_from(rsc_d)

            # ---- G = X^T X partials (per batch) via PE transposes
            g_ps = [pspool.tile([64, 64], f32, tag=f"g{b}", name=f"g_ps{b}")
                    for b in range(B)]
            for b, xp in enumerate([x0, x1]):
                for g in range(8):
                    tp = pspool.tile([128, 512], bf16, tag="tps", bufs=2,
                                     name=f"tp{b}_{g}")
                    for j in range(8):
                        t = g * 8 + j
                        y, xh = divmod(t, 2)
                        off = (y + 1) * WP + 1 + 128 * xh
                        nc.tensor.transpose(tp[:, j * 64:(j + 1) * 64],
                                            xp[0:64, off:off + 128], iden_s[:])
                    xt = xtpool.tile([128, 512], bf16, tag="xt",
                                     name=f"xt{b}_{g}")
                    nc.vector.tensor_copy(xt[:], tp[:])
                    for j in range(8):
                        nc.tensor.matmul(
                            g_ps[b][:],
                            xt[:, j * 64:(j + 1) * 64],
                            xt[:, j * 64:(j + 1) * 64],
                            start=(g == 0 and j == 0),
                            stop=(g == 7 and j == 7),
                            skip_group_check=True,
                        )

            # ---- AllReduce G across the 8 cores
            gcat = spool.tile([64, 128], f32, tag="gcat")
            nc.vector.tensor_copy(gcat[:, 0:64], g_ps[0][:])
            nc.vector.tensor_copy(gcat[:, 64:128], g_ps[1][:])
            g_in = drpool.tile([64, 128], f32, tag="gin")
            g_out = drpool.tile([64, 128], f32, tag="gout")
            nc.sync.dma_start(g_in[:], gcat[:])
            nc.gpsimd.collective_compute(
                "AllReduce", mybir.AluOpType.add,
                replica_groups=[list(range(NCORES))],
                ins=[g_in.opt()], outs=[g_out.opt()],
            )
            gsum = spool.tile([64, 128], f32, tag="gsum")
            nc.sync.dma_start(gsum[:], g_out[:])
            gsum_bf = spool.tile([64, 128], bf16, tag="gsumbf")
            nc.vector.tensor_copy(gsum_bf[:], gsum[:])

            # ---- head math -> Weff per batch, folded into center-tap lhsT
            ctr = []
            gwq_list = []
            nrm_sb = spool.tile([1, 4 * 512], f32, tag="nrmsb")
            for b in range(B):
                gb = gsum_bf[:, b * 64:(b + 1) * 64]

                gwq_ps = pspool.tile([64, 512], f32, tag="tps", bufs=2,
                                     name=f"gwq_ps{b}")
                nc.tensor.matmul(gwq_ps[:], gb, wq_s[:], start=True, stop=True)
                gwq = spool.tile([64, 512], bf16, tag=f"gwq{b}",
                                 name=f"gwq{b}")
                nc.vector.tensor_copy(gwq[:], gwq_ps[:])
                gwq_list.append(gwq)

                gwk_ps = pspool.tile([64, 512], f32, tag="tps", bufs=2,
                                     name=f"gwk_ps{b}")
                nc.tensor.matmul(gwk_ps[:], gb, wk_s[:], start=True, stop=True)
                gwk = spool.tile([64, 512], bf16, tag=f"gwk{b}",
                                 name=f"gwk{b}")
                nc.vector.tensor_copy(gwk[:], gwk_ps[:])

                pk = spool.tile([64, 512], bf16, tag="pk", name=f"pk{b}")
                nc.vector.tensor_mul(pk[:], wk_s[:], gwk[:])
                pq = spool.tile([64, 512], bf16, tag="pq", name=f"pq{b}")
                nc.vector.tensor_mul(pq[:], wq_s[:], gwq[:])

                kk_ps = pspool.tile([1, 512], f32, tag="tps", bufs=2,
                                    name=f"kk_ps{b}")
                nc.tensor.matmul(kk_ps[:], ones_s[:], pk[:], start=True,
                                 stop=True)
                nc.vector.tensor_copy(nrm_sb[0:1, b * 1024:b * 1024 + 512],
                                      kk_ps[:])
                qq_ps = pspool.tile([1, 512], f32, tag="tps", bufs=2,
                                    name=f"qq_ps{b}")
                nc.tensor.matmul(qq_ps[:], ones_s[:], pq[:], start=True,
                                 stop=True)
                nc.vector.tensor_copy(nrm_sb[0:1, b * 1024 + 512:b * 1024 + 1024],
                                      qq_ps[:])

            inv1 = spool.tile([1, 2048], f32, tag="inv1")
            nc.vector.reciprocal(inv1[:], nrm_sb[:])
            inv2 = spool.tile([1, 2048], bf16, tag="inv2")
            nc.scalar.sqrt(inv2[:], inv1[:])  # 1/sqrt(nrm)
            invq = spool.tile([1, 1024], bf16, tag="invq")
            for b in range(B):
                nc.vector.tensor_mul(invq[0:1, b * 512:(b + 1) * 512],
                                     inv2[0:1, b * 1024 + 512:b * 1024 + 1024],
                                     rsc_s[:])

            for b in range(B):
                gwq = gwq_list[b]
                scl_ps = pspool.tile([64, 512], f32, tag="tps", bufs=2,
                                     name=f"scl_ps{b}")
                for h in range(8):
                    nc.tensor.matmul(
                        scl_ps[:, h * 64:(h + 1) * 64],
                        inv2[0:1, b * 1024 + h * 64:b * 1024 + h * 64 + 64],
                        invq[0:1, b * 512 + h * 64:b * 512 + h * 64 + 64],
                        start=True, stop=True, skip_group_check=True,
                    )
                s_ps = pspool.tile([64, 512], f32, tag="tps", bufs=2,
                                   name=f"s_ps{b}")
                for h in range(8):
                    nc.tensor.matmul(
                        s_ps[:, h * 64:(h + 1) * 64],
                        wk_s[:, h * 64:(h + 1) * 64],
                        gwq[:, h * 64:(h + 1) * 64],
                        start=True, stop=True, skip_group_check=True,
                    )
                scl_sb = spool.tile([64, 512], f32, tag="sclsb",
                                    name=f"sclsb{b}")
                nc.vector.tensor_copy(scl_sb[:], scl_ps[:])
                expin = spool.tile([64, 512], f32, tag="expin",
                                   name=f"expin{b}")
                nc.vector.tensor_mul(expin[:], s_ps[:], scl_sb[:])
                attn = spool.tile([64, 512], bf16, tag=f"attn{b}",
                                  name=f"attn{b}")
                nc.scalar.activation(attn[:], expin[:], Act.Exp)
                rs = spool.tile([64, 8], f32, tag="rs", name=f"rs{b}")
                nc.vector.reduce_sum(
                    rs[:], attn[:].rearrange("p (h e) -> p h e", h=8),
                    axis=mybir.AxisListType.X)
                rsi = spool.tile([64, 8], f32, tag="rsi", name=f"rsi{b}")
                nc.vector.reciprocal(rsi[:], rs[:])

                wt_ps = pspool.tile([64, 512], f32, tag="tps", bufs=2,
                                    name=f"wt_ps{b}")
                for h in range(8):
                    wps = spool.tile([64, 64], bf16, tag="wpsc",
                                     name=f"wps{b}_{h}")
                    nc.vector.tensor_scalar_mul(
                        wps[:], wp_s[:, h * 64:(h + 1) * 64], rsi[:, h:h + 1])
                    nc.tensor.matmul(
                        wt_ps[:, h * 64:(h + 1) * 64],
                        attn[:, h * 64:(h + 1) * 64], wps[:],
                        start=True, stop=True, skip_group_check=True,
                    )
                wt_sb = spool.tile([128, 256], bf16, tag="wtsb",
                                   name=f"wtsb{b}")
                for h in range(8):
                    nc.vector.tensor_copy(
                        wt_sb[(h % 2) * 64:(h % 2) * 64 + 64,
                              (h // 2) * 64:(h // 2) * 64 + 64],
                        wt_ps[:, h * 64:(h + 1) * 64])
                weff_ps = pspool.tile([64, 64], f32, tag="tps", bufs=2,
                                      name=f"weff_ps{b}")
                for k in range(4):
                    nc.tensor.matmul(
                        weff_ps[:],
                        wvt_s[:, k * 64:(k + 1) * 64],
                        wt_sb[:, k * 64:(k + 1) * 64],
                        start=(k == 0), stop=(k == 3),
                    )
                weff_bf = spool.tile([128, 64], bf16, tag=f"weffbf{b}",
                                     name=f"weffbf{b}")
                nc.vector.tensor_copy(weff_bf[64:128, :], weff_ps[:])
                c = spool.tile([128, 64], bf16, tag=f"ctr{b}", name=f"ctr{b}")
                nc.vector.tensor_copy(c[0:64, :], taps_s[0:64, 64:128])
                nc.vector.tensor_add(c[64:128, :], taps_s[64:128, 64:128],
                                     weff_bf[64:128, :])
                ctr.append(c)

            # ---- fused conv + attention-output pass
            xv0 = x0[:, :].rearrange("p (r w) -> p r w", w=WP)
            xv1 = x1[:, :].rearrange("p (r w) -> p r w", w=WP)
            for ci in range(NCHUNKS):
                y0 = ci * 2
                cps = pspool.tile([128, 512], f32, tag="conv", bufs=4,
                                  name=f"cps{ci}")
                for b, xv in enumerate([xv0, xv1]):
                    po = 64 * b
                    for dx in range(3):
                        lhs = ctr[b][:] if dx == 1 else \
                            taps_s[:, dx * 64:(dx + 1) * 64]
                        nc.tensor.matmul(
                            cps[po:po + 64, :],
                            lhs,
                            xv[0:128, y0:y0 + 2, dx:dx + 256],
                            start=(dx == 0), stop=False,
                            skip_group_check=True,
                            tile_position=(0, po),
                        )
                    for dx in range(3):
                        nc.tensor.matmul(
                            cps[po:po + 64, :],
                            taps2_s[:, dx * 64:(dx + 1) * 64],
                            xv[0:64, y0 + 2:y0 + 4, dx:dx + 256],
                            start=False, stop=(dx == 2),
                            skip_group_check=True,
                            tile_position=(0, po),
                        )
                gi, gj = divmod(ci, 4)
                if gj == 0:
                    osb = opool.tile([128, 2048], f32, tag="osb",
                                     name=f"osb{gi}")
                nc.scalar.activation(osb[:, gj * 512:(gj + 1) * 512], cps[:],
                                     Act.Identity, bias=bp_s[:])
                if gj == 3:
                    nc.sync.dma_start(
                        out_d[:, gi * 2048:(gi + 1) * 2048], osb[:])

    nc.compile()
    return nc


def _prep_static(Wq, Wk, Wv, rescale, Wp, bp, pos_k):
    pk = np.asarray(pos_k, np.float32).reshape(C, 3, 3)
    eye = np.eye(C, dtype=np.float32)
    taps = np.zeros((128, 192), np.float32)
    taps2 = np.zeros((C, 192), np.float32)
    for dx in range(3):
        taps[0:64, dx * 64:(dx + 1) * 64] = eye * pk[:, 0, dx]
        taps[64:128, dx * 64:(dx + 1) * 64] = eye * pk[:, 1, dx]
        taps2[:, dx * 64:(dx + 1) * 64] = eye * pk[:, 2, dx]
    wvt = np.ascontiguousarray(
        np.asarray(Wv, np.float32).T.reshape(4, 128, 64)
        .transpose(1, 0, 2).reshape(128, 256))
    wp = np.ascontiguousarray(
        np.asarray(Wp, np.float32).reshape(8, 64, 64)
        .transpose(1, 0, 2).reshape(64, 512))
    return {
        "wq": np.asarray(Wq, np.float32).astype(BF),
        "wk": np.asarray(Wk, np.float32).astype(BF),
        "wvt": wvt.astype(BF),
        "wp": wp.astype(np.float32),
        "taps": taps.astype(BF),
        "taps2": taps2.astype(BF),
        "ones": np.ones((C, 1), BF),
        "iden": np.eye(C, dtype=np.float32).astype(BF),
        "bp": np.tile(np.asarray(bp, np.float32), B).reshape(128, 1),
        "rsc": np.repeat(np.asarray(rescale, np.float32).ravel(), 64)
               .reshape(1, INNER).astype(BF),
    }


def _install_ntff_hook():
    """Recreate the antenv.axon_hooks NTFF profiling hook the boot skipped
    (the container's antenv stub lacks axon_hooks).  Profiling only."""
    import sys
    import ctypes
    import contextlib
    import types

    if "antenv.axon_hooks" in sys.modules:
        return
    so_path = "/opt/axon/libaxon_pjrt.so"
    lib = ctypes.CDLL(so_path)
    if not hasattr(lib, "axon_start_nrt_profile"):
        return
    lib.axon_start_nrt_profile.argtypes = [ctypes.POINTER(ctypes.c_int64),
                                           ctypes.c_size_t]
    lib.axon_start_nrt_profile.restype = ctypes.c_int64
    lib.axon_stop_nrt_profile.argtypes = [ctypes.c_char_p]
    lib.axon_stop_nrt_profile.restype = ctypes.c_int64

    @contextlib.contextmanager
    def _hook(output_dir, device_ids):
        import jax
        jax.devices()
        if device_ids:
            ids = (ctypes.c_int64 * len(device_ids))(*device_ids)
            rc = lib.axon_start_nrt_profile(ids, len(device_ids))
        else:
            rc = lib.axon_start_nrt_profile(None, 0)
        if rc != 0:
            raise RuntimeError(f"axon_start_nrt_profile rc={rc}")
        try:
            yield
        finally:
            n = lib.axon_stop_nrt_profile(str(output_dir).encode())
            print(f"profile: {n} ntff file(s) -> {output_dir}")

    mod = types.ModuleType("antenv.axon_hooks")
    mod.get_axon_ntff_profile_hook = lambda: _hook
    mod.set_axon_ntff_profile_hook = lambda h: None
    sys.modules["antenv.axon_hooks"] = mod

    # neutralize the bucket upload (no network share in this container)
    import concourse.bass_utils as bu
    bu.upload_artifacts = lambda tmpdir: tmpdir


def kernel(x_in, Wq, Wk, Wv, rescale, Wp, bp, pos_k):
    from concourse.bass_utils import run_bass_kernel_spmd

    if "nc" not in _CACHE:
        _CACHE["nc"] = _build()
    nc = _CACHE["nc"]

    x_in = np.asarray(x_in, np.float32)
    static = _prep_static(Wq, Wk, Wv, rescale, Wp, bp, pos_k)

    xp = np.zeros((B, C, H + 2, W + 2), np.float32)
    xp[:, :, 1:H + 1, 1:W + 1] = x_in
    in_maps = []
    for i in range(NCORES):
        shard = np.ascontiguousarray(
            xp[:, :, i * RPC:i * RPC + HP, :]).reshape(B * C, FREE)
        in_maps.append({"x": shard, **static})

    trace = os.environ.get("KERNEL_PROFILE", "0") == "1"
    if trace:
        try:
            _install_ntff_hook()
        except Exception as e:
            print(f"ntff hook install failed: {e}")
            trace = False
    tmpdir = os.environ.get("KERNEL_TRACE_DIR") or None
    res = run_bass_kernel_spmd(nc, in_maps, core_ids=list(range(NCORES)),
                               trace=trace, tmpdir=tmpdir)
    _CACHE["exec_time_ns"] = res.exec_time_ns

    out = np.empty((B, C, H, W), np.float32)
    for i in range(NCORES):
        o = np.asarray(res.results[i]["out"], np.float32).reshape(B, C, RPC, W)
        out[:, :, i * RPC:(i + 1) * RPC, :] = o
    return out


# revision 13
# speedup vs baseline: 1.3593x; 1.3593x over previous
"""Distributed Trainium2 kernel for the sparse-attention + depthwise-conv module.

Math: q/k are l2-normalized over the full spatial axis n and the score matrix
is a tiny [b,h,64,64], so the whole attention collapses through the per-batch
Gram matrix G = X^T X ([64,64]):
  S_raw[h] = Wk_h^T G Wq_h, kk = diag(Wk_h^T G Wk_h), qq = diag(Wq_h^T G Wq_h)
  attn = softmax(S_raw * rescale / sqrt(kk qq))
  Wtilde[h] = attn_h^T (Wp_h / rowsum),  Weff = Wv @ Wtilde   ([64,64] per b)
  out = depthwise_conv3x3(x) + X @ Weff + bp
Only G crosses cores (AllReduce of 2*64*64 f32 = 32KB).

Sharding: 256 rows split into 8 slabs of 32 rows (halo pre-padded host-side),
both batches on every core.  x lives in SBUF as bf16 [128, 34*258] per batch
with a one-row-shifted copy in partitions 64:127, which serves double duty:
 - conv taps (dy,dx),(dy+1,dx) pack into one K=128 matmul
 - G transposes lift two image rows per PE op ([128,128] matmul vs identity)
The conv+attention output pass accumulates 6 matmul slots per 512-col chunk;
the two batches run concurrently in opposite PE column groups writing to
separate PSUM banks.  The attention term (X @ Weff) is applied as a second
accumulation generation so all conv work overlaps the AllReduce+head-math
latency.
"""

import os
import numpy as np
import ml_dtypes

BF = ml_dtypes.bfloat16
B, C, H, W = 2, 64, 256, 256
HEADS, D = 8, 64
INNER = HEADS * D          # 512
NCORES = 8
RPC = H // NCORES          # 32 output rows per core per batch
WP = W + 2                 # 258 padded row length
HP = RPC + 2               # 34 rows incl halo
FREE = HP * WP             # 8772
SHIFT_FREE = FREE - WP     # 8514
NLOC = RPC * W             # 8192 spatial positions per core per batch
NCHUNKS = NLOC // 512      # 16

_CACHE = {}


def _build():
    import concourse.bass as bass
    import concourse.bacc as bacc
    import concourse.mybir as mybir
    import concourse.tile as tile

    f32 = mybir.dt.float32
    bf16 = mybir.dt.bfloat16

    nc = bacc.Bacc("TRN2", target_bir_lowering=False, debug=False,
                   num_devices=NCORES)

    x_d = nc.dram_tensor("x", [B * C, FREE], f32, kind="ExternalInput").ap()
    wq_d = nc.dram_tensor("wq", [C, INNER], bf16, kind="ExternalInput").ap()
    wk_d = nc.dram_tensor("wk", [C, INNER], bf16, kind="ExternalInput").ap()
    wvt_d = nc.dram_tensor("wvt", [128, 256], bf16, kind="ExternalInput").ap()
    wp_d = nc.dram_tensor("wp", [D, INNER], f32, kind="ExternalInput").ap()
    taps_d = nc.dram_tensor("taps", [128, 192], bf16, kind="ExternalInput").ap()
    taps2_d = nc.dram_tensor("taps2", [C, 192], bf16, kind="ExternalInput").ap()
    ctrb_d = nc.dram_tensor("ctrb", [128, 64], bf16, kind="ExternalInput").ap()
    ones_d = nc.dram_tensor("ones", [C, 1], bf16, kind="ExternalInput").ap()
    idn_d = nc.dram_tensor("idn", [128, 128], bf16, kind="ExternalInput").ap()
    bp_d = nc.dram_tensor("bp", [128, 1], f32, kind="ExternalInput").ap()
    rsc_d = nc.dram_tensor("rsc", [1, INNER], bf16, kind="ExternalInput").ap()
    out_d = nc.dram_tensor("out", [B * C, NLOC], f32, kind="ExternalOutput").ap()

    Act = mybir.ActivationFunctionType
    N_EARLY = int(os.environ.get("KERNEL_EARLY_PAIRS", "16"))  # gen2 pairs

    with tile.TileContext(nc) as tc:
        with (
            tc.tile_pool(name="xp", bufs=1) as xpool,
            tc.tile_pool(name="wp", bufs=1) as wpool,
            tc.tile_pool(name="sp", bufs=1) as spool,
            tc.tile_pool(name="xt", bufs=3) as xtpool,
            tc.tile_pool(name="ob", bufs=4) as opool,
            tc.tile_pool(name="ps", bufs=1, space="PSUM") as pspool,
            tc.tile_pool(name="dr", bufs=1, space="DRAM") as drpool,
        ):
            # ---- load x (cast f32->bf16 in flight), build row-shifted copies
            x0 = xpool.tile([128, FREE], bf16, tag="x0")
            x1 = xpool.tile([128, FREE], bf16, tag="x1")
            hf = FREE // 2
            nc.gpsimd.dma_start(x0[0:64, 0:hf], x_d[0:64, 0:hf])
            nc.gpsimd.dma_start(x0[0:64, hf:FREE], x_d[0:64, hf:FREE])
            nc.gpsimd.dma_start(x1[0:64, 0:hf], x_d[64:128, 0:hf])
            nc.gpsimd.dma_start(x1[0:64, hf:FREE], x_d[64:128, hf:FREE])
            nc.vector.tensor_copy(x0[64:128, 0:SHIFT_FREE], x0[0:64, WP:FREE])
            nc.vector.tensor_copy(x1[64:128, 0:SHIFT_FREE], x1[0:64, WP:FREE])

            # ---- weights
            wq_s = wpool.tile_from(wq_d)
            wk_s = wpool.tile_from(wk_d)
            wvt_s = wpool.tile_from(wvt_d)
            wp_s = wpool.tile_from(wp_d)
            taps_s = wpool.tile_from(taps_d)
            taps2_s = wpool.tile_from(taps2_d)
            ctrb_s = wpool.tile_from(ctrb_d)
            ones_s = wpool.tile_from(ones_d)
            idn_s = wpool.tile_from(idn_d)
            bp_s = wpool.tile_from(bp_d)
            rsc_s = wpool.tile_from(rsc_d)

            # ---- G = X^T X partials per batch.
            # Pair-transpose: lhsT = x[:, off:off+128] ([128part=(ch,row/row+1),
            # 128 cols]) against I128 -> psum [128 cols, 128 (ch_y|ch_y1)].
            # Each yields two K=128 G-matmuls (col-half = one image row).
            g_ps = [pspool.tile([64, 64], f32, tag=f"g{b}", name=f"g_ps{b}")
                    for b in range(B)]
            for b, xp in enumerate([x0, x1]):
                first = True
                for grp in range(8):    # 4 pair-tiles per psum bank
                    tp = pspool.tile([128, 512], f32, tag="tps", bufs=2,
                                     name=f"tp{b}_{grp}")
                    for j in range(4):
                        t = grp * 4 + j          # 0..31
                        y, xh = divmod(t, 2)     # y-pair index 0..15, half
                        off = (2 * y + 1) * WP + 1 + 128 * xh
                        nc.tensor.matmul(tp[:, j * 128:(j + 1) * 128],
                                         xp[0:128, off:off + 128], idn_s[:],
                                         start=True, stop=True,
                                         skip_group_check=True)
                    xt = xtpool.tile([128, 512], bf16, tag="xt",
                                     name=f"xt{b}_{grp}")
                    nc.vector.tensor_copy(xt[:], tp[:])
                    for j in range(8):
                        nc.tensor.matmul(
                            g_ps[b][:],
                            xt[:, j * 64:(j + 1) * 64],
                            xt[:, j * 64:(j + 1) * 64],
                            start=first, stop=(grp == 7 and j == 7),
                            skip_group_check=True,
                        )
                        first = False

            # ---- AllReduce G across the 8 cores
            gcat = spool.tile([64, 128], f32, tag="gcat")
            nc.vector.tensor_copy(gcat[:, 0:64], g_ps[0][:])
            nc.vector.tensor_copy(gcat[:, 64:128], g_ps[1][:])
            g_in = drpool.tile([64, 128], f32, tag="gin")
            g_out = drpool.tile([64, 128], f32, tag="gout")
            nc.sync.dma_start(g_in[:], gcat[:])
            nc.gpsimd.collective_compute(
                "AllReduce", mybir.AluOpType.add,
                replica_groups=[list(range(NCORES))],
                ins=[g_in.opt()], outs=[g_out.opt()],
            )
            gsum = spool.tile([64, 128], f32, tag="gsum")
            nc.sync.dma_start(gsum[:], g_out[:])
            gsum_bf = spool.tile([64, 128], bf16, tag="gsumbf")
            nc.vector.tensor_copy(gsum_bf[:], gsum[:])

            # ---- head math -> Weff per batch (tiny, PE+DVE+ACT)
            ctr = []
            gwq_list = []
            nrm_sb = spool.tile([1, 4 * 512], f32, tag="nrmsb")
            for b in range(B):
                gb = gsum_bf[:, b * 64:(b + 1) * 64]
                gwk_ps = pspool.tile([64, 512], f32, tag="tps", bufs=2,
                                     name=f"gwk_ps{b}")
                nc.tensor.matmul(gwk_ps[:], gb, wk_s[:], start=True, stop=True)
                pk = spool.tile([64, 512], bf16, tag=f"pk{b}", name=f"pk{b}")
                nc.vector.tensor_mul(pk[:], wk_s[:], gwk_ps[:])
                gwq_ps = pspool.tile([64, 512], f32, tag="tps", bufs=2,
                                     name=f"gwq_ps{b}")
                nc.tensor.matmul(gwq_ps[:], gb, wq_s[:], start=True, stop=True)
                pq = spool.tile([64, 512], bf16, tag=f"pq{b}", name=f"pq{b}")
                nc.vector.tensor_mul(pq[:], wq_s[:], gwq_ps[:])
                gwq = spool.tile([64, 512], bf16, tag=f"gwq{b}",
                                 name=f"gwq{b}")
                nc.vector.tensor_copy(gwq[:], gwq_ps[:])
                gwq_list.append(gwq)

                kk_ps = pspool.tile([1, 512], f32, tag="tps", bufs=2,
                                    name=f"kk_ps{b}")
                nc.tensor.matmul(kk_ps[:], ones_s[:], pk[:], start=True,
                                 stop=True)
                nc.vector.tensor_copy(nrm_sb[0:1, b * 1024:b * 1024 + 512],
                                      kk_ps[:])
                qq_ps = pspool.tile([1, 512], f32, tag="tps", bufs=2,
                                    name=f"qq_ps{b}")
                nc.tensor.matmul(qq_ps[:], ones_s[:], pq[:], start=True,
                                 stop=True)
                nc.vector.tensor_copy(
                    nrm_sb[0:1, b * 1024 + 512:b * 1024 + 1024], qq_ps[:])

            inv1 = spool.tile([1, 2048], f32, tag="inv1")
            nc.vector.reciprocal(inv1[:], nrm_sb[:])
            inv2 = spool.tile([1, 2048], bf16, tag="inv2")
            nc.scalar.sqrt(inv2[:], inv1[:])  # 1/sqrt(nrm)
            invq = spool.tile([1, 1024], bf16, tag="invq")
            for b in range(B):
                nc.vector.tensor_mul(invq[0:1, b * 512:(b + 1) * 512],
                                     inv2[0:1, b * 1024 + 512:b * 1024 + 1024],
                                     rsc_s[:])

            for b in range(B):
                gwq = gwq_list[b]
                scl_ps = pspool.tile([64, 512], f32, tag="tps", bufs=2,
                                     name=f"scl_ps{b}")
                for h in range(8):
                    nc.tensor.matmul(
                        scl_ps[:, h * 64:(h + 1) * 64],
                        inv2[0:1, b * 1024 + h * 64:b * 1024 + h * 64 + 64],
                        invq[0:1, b * 512 + h * 64:b * 512 + h * 64 + 64],
                        start=True, stop=True, skip_group_check=True,
                    )
                scl_sb = spool.tile([64, 512], f32, tag="sclsb",
                                    name=f"sclsb{b}")
                nc.vector.tensor_copy(scl_sb[:], scl_ps[:])
                s_ps = pspool.tile([64, 512], f32, tag="tps", bufs=2,
                                   name=f"s_ps{b}")
                for h in range(8):
                    nc.tensor.matmul(
                        s_ps[:, h * 64:(h + 1) * 64],
                        wk_s[:, h * 64:(h + 1) * 64],
                        gwq[:, h * 64:(h + 1) * 64],
                        start=True, stop=True, skip_group_check=True,
                    )
                expin = spool.tile([64, 512], f32, tag="expin",
                                   name=f"expin{b}")
                nc.vector.tensor_mul(expin[:], s_ps[:], scl_sb[:])
                attn = spool.tile([64, 512], bf16, tag=f"attn{b}",
                                  name=f"attn{b}")
                nc.scalar.activation(attn[:], expin[:], Act.Exp)
                rs = spool.tile([64, 8], f32, tag="rs", name=f"rs{b}")
                nc.vector.reduce_sum(
                    rs[:], attn[:].rearrange("p (h e) -> p h e", h=8),
                    axis=mybir.AxisListType.X)
                rsi = spool.tile([64, 8], f32, tag="rsi", name=f"rsi{b}")
                nc.vector.reciprocal(rsi[:], rs[:])

                wt_ps = pspool.tile([64, 512], f32, tag="tps", bufs=2,
                                    name=f"wt_ps{b}")
                for h in range(8):
                    wps = spool.tile([64, 64], bf16, tag="wpsc",
                                     name=f"wps{b}_{h}")
                    nc.vector.tensor_scalar_mul(
                        wps[:], wp_s[:, h * 64:(h + 1) * 64], rsi[:, h:h + 1])
                    nc.tensor.matmul(
                        wt_ps[:, h * 64:(h + 1) * 64],
                        attn[:, h * 64:(h + 1) * 64], wps[:],
                        start=True, stop=True, skip_group_check=True,
                    )
                wt_sb = spool.tile([128, 256], bf16, tag="wtsb",
                                   name=f"wtsb{b}")
                for h in range(8):
                    nc.vector.tensor_copy(
                        wt_sb[(h % 2) * 64:(h % 2) * 64 + 64,
                              (h // 2) * 64:(h // 2) * 64 + 64],
                        wt_ps[:, h * 64:(h + 1) * 64])
                weff_ps = pspool.tile([64, 64], f32, tag="tps", bufs=2,
                                      name=f"weff_ps{b}")
                for k in range(4):
                    nc.tensor.matmul(
                        weff_ps[:],
                        wvt_s[:, k * 64:(k + 1) * 64],
                        wt_sb[:, k * 64:(k + 1) * 64],
                        start=(k == 0), stop=(k == 3),
                    )
                # ctr lhsT: rows 0:64 = Weff_b (pairs with unshifted center
                # sample), rows 64:128 = diag(pos_k[2,1]) (shifted copy hits
                # row y+2, col+1).  Used by the folded (late) chunk pairs.
                c = spool.tile([128, 64], bf16, tag=f"ctr{b}", name=f"ctr{b}")
                nc.vector.tensor_copy(c[0:64, :], weff_ps[:])
                nc.vector.tensor_copy(c[64:128, :], ctrb_s[64:128, :])
                ctr.append(c)

            # ---- fused conv (+attention for folded pairs) main pass
            # per chunk-pair: b0 -> cps0[0:64] (col groups 0-1), b1 ->
            # cps1[64:128] (col groups 2-3); interleaved issue so the two
            # batches' matmuls run concurrently in opposite array halves.
            xv0 = x0[:, :].rearrange("p (r w) -> p r w", w=WP)
            xv1 = x1[:, :].rearrange("p (r w) -> p r w", w=WP)
            xvs = [xv0, xv1]

            def conv_slots(ci, folded):
                """[(lhsT_b0, lhsT_b1, part_lo, part_hi, row_off, col_off)]"""
                sl = []
                for dx in range(3):
                    t = taps_s[:, dx * 64:(dx + 1) * 64]
                    if folded and dx == 1:
                        # conv pair (0,1),(1,1) stays; Weff handled below
                        pass
                    sl.append((t, t, 0, 128, 0, dx))
                for dx in range(3):
                    if folded and dx == 1:
                        continue
                    t2 = taps2_s[:, dx * 64:(dx + 1) * 64]
                    sl.append((t2, t2, 0, 64, 2, dx))
                if folded:
                    # pair [Weff ; diag(2,1)]: unshifted half reads the center
                    # sample, shifted half reads row y+2 col 1
                    sl.append((ctr[0], ctr[1], 0, 128, 1, 1))
                return sl

            osbs = {}
            early = set(range(NCHUNKS - N_EARLY, NCHUNKS))

            def gen1(ci):
                folded = ci not in early
                y0 = ci * 2
                cps0 = pspool.tile([128, 512], f32, tag="conv", bufs=4,
                                   name=f"cps0_{ci}")
                cps1 = pspool.tile([128, 512], f32, tag="conv", bufs=4,
                                   name=f"cps1_{ci}")
                slots = conv_slots(ci, folded)
                nsl = len(slots)
                for si, (t0_, t1_, plo, phi, dy, dx) in enumerate(slots):
                    st, sp = (si == 0), (si == nsl - 1)
                    nc.tensor.matmul(
                        cps0[0:64, :], t0_[plo:phi, :],
                        xvs[0][plo:phi, y0 + dy:y0 + dy + 2, dx:dx + 256],
                        start=st, stop=sp, skip_group_check=True,
                        tile_position=(0, 0))
                    nc.tensor.matmul(
                        cps1[64:128, :], t1_[plo:phi, :],
                        xvs[1][plo:phi, y0 + dy:y0 + dy + 2, dx:dx + 256],
                        start=st, stop=sp, skip_group_check=True,
                        tile_position=(0, 64))
                gi, gj = divmod(ci, 4)
                if gi not in osbs:
                    osbs[gi] = opool.tile([128, 2048], f32, tag="osb",
                                          name=f"osb{gi}")
                osb = osbs[gi]
                nc.scalar.activation(osb[0:64, gj * 512:(gj + 1) * 512],
                                     cps0[0:64, :], Act.Identity,
                                     bias=bp_s[0:64, :])
                nc.scalar.activation(osb[64:128, gj * 512:(gj + 1) * 512],
                                     cps1[64:128, :], Act.Identity,
                                     bias=bp_s[64:128, :])

            def gen2(ci):
                # X @ Weff for the early (unfolded) pairs: lhsT at row base 64
                # so rhs reads the shifted copy; offset y0 row -> center row.
                y0 = ci * 2
                aps0 = pspool.tile([128, 512], f32, tag="tps", bufs=2,
                                   name=f"aps0_{ci}")
                aps1 = pspool.tile([128, 512], f32, tag="tps", bufs=2,
                                   name=f"aps1_{ci}")
                nc.tensor.matmul(aps0[0:64, :], ctr[0][0:64, :],
                                 xvs[0][0:64, y0 + 1:y0 + 3, 1:257],
                                 start=True, stop=True, skip_group_check=True,
                                 tile_position=(0, 0))
                nc.tensor.matmul(aps1[64:128, :], ctr[1][0:64, :],
                                 xvs[1][0:64, y0 + 1:y0 + 3, 1:257],
                                 start=True, stop=True, skip_group_check=True,
                                 tile_position=(0, 64))
                gi, gj = divmod(ci, 4)
                osb = osbs[gi]
                sl = osb[0:64, gj * 512:(gj + 1) * 512]
                nc.vector.tensor_add(sl, sl, aps0[0:64, :])
                sl = osb[64:128, gj * 512:(gj + 1) * 512]
                nc.vector.tensor_add(sl, sl, aps1[64:128, :])

            done_groups = set()

            def flush(gi):
                if gi in done_groups:
                    return
                done_groups.add(gi)
                nc.gpsimd.dma_start(out_d[:, gi * 2048:(gi + 1) * 2048],
                                    osbs[gi][:])

            # early (AR-independent) pairs first in program order so the PE
            # stream never stalls on the AllReduce before reaching them
            order = sorted(early) + [c for c in range(NCHUNKS)
                                     if c not in early]
            gen1_done = set()
            for ci in order:
                gen1(ci)
                gen1_done.add(ci)
                gi = ci // 4
                grp = set(range(gi * 4, gi * 4 + 4))
                if grp <= gen1_done and not (grp & early):
                    flush(gi)
            gen2_done = set()
            for ci in sorted(early):
                gen2(ci)
                gen2_done.add(ci)
                gi = ci // 4
                grp = set(range(gi * 4, gi * 4 + 4))
                if grp <= (gen1_done | set()) and (grp & early) <= gen2_done:
                    flush(gi)
            for gi in range(NCHUNKS // 4):
                flush(gi)

    nc.compile()
    return nc


def _prep_static(Wq, Wk, Wv, rescale, Wp, bp, pos_k):
    pk = np.asarray(pos_k, np.float32).reshape(C, 3, 3)
    eye = np.eye(C, dtype=np.float32)
    taps = np.zeros((128, 192), np.float32)
    taps2 = np.zeros((C, 192), np.float32)
    ctrb = np.zeros((128, 64), np.float32)
    for dx in range(3):
        taps[0:64, dx * 64:(dx + 1) * 64] = eye * pk[:, 0, dx]
        taps[64:128, dx * 64:(dx + 1) * 64] = eye * pk[:, 1, dx]
        taps2[:, dx * 64:(dx + 1) * 64] = eye * pk[:, 2, dx]
    ctrb[64:128, :] = eye * pk[:, 2, 1]
    wvt = np.ascontiguousarray(
        np.asarray(Wv, np.float32).T.reshape(4, 128, 64)
        .transpose(1, 0, 2).reshape(128, 256))
    wp = np.ascontiguousarray(
        np.asarray(Wp, np.float32).reshape(8, 64, 64)
        .transpose(1, 0, 2).reshape(64, 512))
    return {
        "wq": np.asarray(Wq, np.float32).astype(BF),
        "wk": np.asarray(Wk, np.float32).astype(BF),
        "wvt": wvt.astype(BF),
        "wp": wp.astype(np.float32),
        "taps": taps.astype(BF),
        "taps2": taps2.astype(BF),
        "ctrb": ctrb.astype(BF),
        "ones": np.ones((C, 1), BF),
        "idn": np.eye(128, dtype=np.float32).astype(BF),
        "bp": np.tile(np.asarray(bp, np.float32), B).reshape(128, 1),
        "rsc": np.repeat(np.asarray(rescale, np.float32).ravel(), 64)
               .reshape(1, INNER).astype(BF),
    }


def _install_ntff_hook():
    """Recreate the antenv.axon_hooks NTFF profiling hook the boot skipped
    (the container's antenv stub lacks axon_hooks).  Profiling only."""
    import sys
    import ctypes
    import contextlib
    import types

    if "antenv.axon_hooks" in sys.modules:
        return
    so_path = "/opt/axon/libaxon_pjrt.so"
    lib = ctypes.CDLL(so_path)
    if not hasattr(lib, "axon_start_nrt_profile"):
        return
    lib.axon_start_nrt_profile.argtypes = [ctypes.POINTER(ctypes.c_int64),
                                           ctypes.c_size_t]
    lib.axon_start_nrt_profile.restype = ctypes.c_int64
    lib.axon_stop_nrt_profile.argtypes = [ctypes.c_char_p]
    lib.axon_stop_nrt_profile.restype = ctypes.c_int64

    @contextlib.contextmanager
    def _hook(output_dir, device_ids):
        import jax
        jax.devices()
        if device_ids:
            ids = (ctypes.c_int64 * len(device_ids))(*device_ids)
            rc = lib.axon_start_nrt_profile(ids, len(device_ids))
        else:
            rc = lib.axon_start_nrt_profile(None, 0)
        if rc != 0:
            raise RuntimeError(f"axon_start_nrt_profile rc={rc}")
        try:
            yield
        finally:
            n = lib.axon_stop_nrt_profile(str(output_dir).encode())
            print(f"profile: {n} ntff file(s) -> {output_dir}")

    mod = types.ModuleType("antenv.axon_hooks")
    mod.get_axon_ntff_profile_hook = lambda: _hook
    mod.set_axon_ntff_profile_hook = lambda h: None
    sys.modules["antenv.axon_hooks"] = mod

    import concourse.bass_utils as bu
    bu.upload_artifacts = lambda tmpdir: tmpdir


def kernel(x_in, Wq, Wk, Wv, rescale, Wp, bp, pos_k):
    from concourse.bass_utils import run_bass_kernel_spmd

    if "nc" not in _CACHE:
        _CACHE["nc"] = _build()
    nc = _CACHE["nc"]

    x_in = np.asarray(x_in, np.float32)
    static = _prep_static(Wq, Wk, Wv, rescale, Wp, bp, pos_k)

    xp = np.zeros((B, C, H + 2, W + 2), np.float32)
    xp[:, :, 1:H + 1, 1:W + 1] = x_in
    in_maps = []
    for i in range(NCORES):
        shard = np.ascontiguousarray(
            xp[:, :, i * RPC:i * RPC + HP, :]).reshape(B * C, FREE)
        in_maps.append({"x": shard, **static})

    trace = os.environ.get("KERNEL_PROFILE", "0") == "1"
    if trace:
        try:
            _install_ntff_hook()
        except Exception as e:
            print(f"ntff hook install failed: {e}")
            trace = False
    tmpdir = os.environ.get("KERNEL_TRACE_DIR") or None
    res = run_bass_kernel_spmd(nc, in_maps, core_ids=list(range(NCORES)),
                               trace=trace, tmpdir=tmpdir)
    _CACHE["exec_time_ns"] = res.exec_time_ns

    out = np.empty((B, C, H, W), np.float32)
    for i in range(NCORES):
        o = np.asarray(res.results[i]["out"], np.float32).reshape(B, C, RPC, W)
        out[:, :, i * RPC:(i + 1) * RPC, :] = o
    return out
